# revision 39
# baseline (speedup 1.0000x reference)
"""Trainium2 Bass kernel for AttentionBilinear.

Per batch b:
    pW     = p[b] @ W                         # [Tp, Dq]
    scores = pW @ q[b].T                      # [Tp, Tq]
    wts    = softmax(scores, axis=Tp)
    out[b] = wts @ q[b]                       # [Tp, Dq]

Computed in the transposed-scores orientation so the softmax over Tp is a
free-axis reduction:
    pWT[d, tp]  = sum_e W[e, d] * pT[e, tp]       (mm1: lhsT=W,  rhs=pT)
    scT[tq, tp] = sum_d qT[d, tq] * pWT[d, tp]    (mm2: lhsT=qT, rhs=pWT)
    softmax over tp (free axis), read from PSUM   (DVE max / ACT exp / DVE mul)
    out[tp, d]  = sum_tq wT[tq, tp] * q[tq, d]    (mm3: lhsT=wT, rhs=q)

fp32 matmul on TRN2 runs at 4 cycles/row; fp16 at 1 cycle/row with an 11-bit
mantissa (measured end-to-end relative error ~5e-3 vs the fp32 reference).
The host pre-shards per core and pre-packs layouts: fp16 casts and the
q/p transposes are host-side layout prep, so the device runs a pure
matmul + softmax pipeline.

Modes per (mm1, mm2):
    'hi'    — single fp16 matmul on rounded inputs
    'split' — hi/lo decomposition A@B ~ Ah@Bh + Ah@Bl + Al@Bh (error ~1e-5),
              3x the matmul work.
mm3 always runs 1-term fp16 (its operands are smooth; error ~3e-4).

Sharding: data-parallel over batch B=16 across 8 cores, W replicated.
"""

import numpy as np

P = 128   # partitions
H = 512   # PSUM bank width in fp32

B_FULL = 16
T_FULL = 1024
D_FULL = 1024
N_CORES = 8

MODE = ("hi", "hi")  # (mm1, mm2)


def build_nc(b_loc=2, t=1024, d=1024, mode=MODE):
    from contextlib import ExitStack

    import concourse.tile as tile
    from concourse import bacc, mybir

    f32 = mybir.dt.float32
    f16 = mybir.dt.float16
    C = t // P     # row chunks of a [t, d] matrix
    KC = d // P    # chunks of the d (feature) axis
    TH = t // H    # 512-wide pieces of the t axis
    NH = d // H    # 512-wide pieces of the d axis
    AX = mybir.AxisListType.X
    EXP = mybir.ActivationFunctionType.Exp
    MIN = mybir.AluOpType.min
    ADD = mybir.AluOpType.add
    SUB = mybir.AluOpType.subtract
    m1, m2 = mode
    split1 = m1 == "split"
    split2 = m2 == "split"

    nc = bacc.Bacc()

    # Inputs are host-packed partition-major ([b, p, c, cols]) so every DMA
    # descriptor is an 8-16KB contiguous run (vs 1-2KB row-major).
    def dram_in(name):
        return nc.dram_tensor(
            name, [b_loc, P, C, d], f16, kind="ExternalInput"
        ).ap()

    qh_ext = dram_in("qh")          # q natural, fp16
    qt_ext = dram_in("qt")          # q transposed per batch: [d, tq]
    # p transposed, additionally split into tp-halves: [b, TH, p, c, H]
    pt_ext = nc.dram_tensor(
        "pt", [b_loc, TH, P, C, H], f16, kind="ExternalInput"
    ).ap()
    # W host-blocked as [m, p, ce, c] = W[ce*128+p, m*128+c] so each m-piece
    # is one contiguous 256KB DMA.
    w_ext = nc.dram_tensor("w", [KC, P, KC, P], f16, kind="ExternalInput").ap()
    qtl_ext = dram_in("qtl") if split2 else None
    ptl_ext = dram_in("ptl") if split1 else None
    wl_ext = (
        nc.dram_tensor("wl", [KC, P, KC, P], f16, kind="ExternalInput").ap()
        if split1
        else None
    )
    out_ext = nc.dram_tensor("out", [b_loc, t, d], f32, kind="ExternalOutput").ap()

    big_bufs = 1 if (split1 or split2) else 2

    with tile.TileContext(nc) as tc, ExitStack() as ctx:
        consts = ctx.enter_context(tc.tile_pool(name="consts", bufs=1))
        qh_pool = ctx.enter_context(tc.tile_pool(name="qh_pool", bufs=2))
        qt_pool = ctx.enter_context(tc.tile_pool(name="qt_pool", bufs=big_bufs))
        pt_pool = ctx.enter_context(tc.tile_pool(name="pt_pool", bufs=big_bufs))
        pwt_pool = ctx.enter_context(tc.tile_pool(name="pwt_pool", bufs=big_bufs))
        wt_pool = ctx.enter_context(tc.tile_pool(name="wt_pool", bufs=big_bufs))
        ostage = ctx.enter_context(tc.tile_pool(name="ostage", bufs=4))
        stats = ctx.enter_context(tc.tile_pool(name="stats", bufs=2))
        psum_mm = ctx.enter_context(tc.tile_pool(name="psum_mm", bufs=8, space="PSUM"))

        # ---- PE warm-up: ~16 junk matmuls while the first DMAs land, so the
        # HAM clock-gate is already released when real work starts ----
        warm = consts.tile([P, H], f16, name="warm")
        nc.gpsimd.memset(warm[:], 0.0)
        wacc = psum_mm.tile([P, H], f32, name="wacc", tag="acc")
        for i in range(10):
            nc.tensor.matmul(
                wacc[:], warm[:, 0:P], warm[:], start=(i == 0), stop=(i == 9)
            )

        # ---- W resident (fp16), blocked [p, m, ce, c] ----
        # Loaded in m-pieces (each one contiguous 256KB DMA) on the scalar
        # ring, which is idle during startup — W streams in parallel with
        # pT's first half on the sync ring, so mm1 starts ~4us earlier.
        # (The ACT sequencer is done issuing these long before the first
        # PSUM-drain copy at ~18us.) lhsT slice for (k=ce, m) is w[:, m, k, :].
        def load_w(name, ext):
            wt_ = consts.tile([P, KC, KC, P], f16, name=name)
            for m in range(KC):
                eng = nc.scalar if m % 2 == 0 else nc.sync
                eng.dma_start(wt_[:, m], ext[m])
            return wt_

        globals_w = {}

        st = [dict() for _ in range(b_loc)]

        def load_mat(pool, name, tag, ext, b, engine):
            """packed [b, P, C, d] DRAM (fp16) -> [P, C, d] SBUF in one DMA."""
            mt = pool.tile([P, C, d], f16, name=name, tag=tag)
            engine.dma_start(mt[:], ext[b])
            return mt

        def load_pt(b, engine):
            mt = pt_pool.tile([P, C, d], f16, name=f"pT_{b}", tag="pT")
            for h in range(TH):
                engine.dma_start(mt[:, :, h * H : (h + 1) * H], pt_ext[b, h])
            return mt

        def phase_loads(b):
            # All loads ride the sync ring as one FIFO in exact consumption
            # order, so early phases are never starved by later tensors.
            if b == 0:
                # Batch 0 startup: pT half 0 (sync) runs in parallel with the
                # W pieces (scalar ring), then pT half 1.
                pt0 = pt_pool.tile([P, C, d], f16, name="pT_0", tag="pT")
                nc.sync.dma_start(pt0[:, :, 0:H], pt_ext[0, 0])
                globals_w["w_hi"] = load_w("w_hi", w_ext)
                if split1:
                    globals_w["w_lo"] = load_w("w_lo", wl_ext)
                for h in range(1, TH):
                    nc.sync.dma_start(
                        pt0[:, :, h * H : (h + 1) * H], pt_ext[0, h]
                    )
                st[0]["pT"] = pt0
            else:
                st[b]["pT"] = load_pt(b, nc.sync)
            if split1:
                st[b]["pTl"] = load_mat(pt_pool, f"pTl_{b}", "pTl", ptl_ext, b, nc.sync)
            st[b]["qT"] = load_mat(qt_pool, f"qT_{b}", "qT", qt_ext, b, nc.sync)
            if split2:
                st[b]["qTl"] = load_mat(qt_pool, f"qTl_{b}", "qTl", qtl_ext, b, nc.sync)
            st[b]["qh"] = load_mat(qh_pool, f"qh_{b}", "qh", qh_ext, b, nc.sync)

        def mm_terms(acc, terms, n_sl):
            """terms: list of (lhs_slicer(k) -> AP[128,128], rhs_mat)."""
            n_inst = len(terms) * KC
            i = 0
            for lhs_of, rhs_mat in terms:
                for k in range(KC):
                    nc.tensor.matmul(
                        acc[:],
                        lhs_of(k),
                        rhs_mat[:, k, n_sl],
                        start=(i == 0),
                        stop=(i == n_inst - 1),
                    )
                    i += 1

        def phase_mm1(b):
            """pWT[d, tp] = sum_e W[e,d] * pT[e,tp]."""
            pT = st[b]["pT"]
            pWT = pwt_pool.tile([P, KC, t], f16, name=f"pWT_{b}", tag="pWT")
            pWTl = (
                pwt_pool.tile([P, KC, t], f16, name=f"pWTl_{b}", tag="pWTl")
                if split2
                else None
            )
            for n in range(TH):  # n outer: start on pT's first half early
                n_sl = slice(n * H, (n + 1) * H)
                for m in range(KC):
                    msl = slice(m * P, (m + 1) * P)

                    def w_sl(k, m=m, w=globals_w["w_hi"]):
                        return w[:, m, k, :]

                    acc = psum_mm.tile([P, H], f32, name=f"a1_{b}_{m}_{n}", tag="acc")
                    terms = [(w_sl, pT)]
                    if split1:
                        terms += [
                            (w_sl, st[b]["pTl"]),
                            (lambda k, m=m, w=globals_w["w_lo"]: w[:, m, k, :], pT),
                        ]
                    mm_terms(acc, terms, n_sl)
                    nc.scalar.copy(pWT[:, m, n_sl], acc[:])
                    if pWTl is not None:
                        nc.vector.tensor_tensor(
                            pWTl[:, m, n_sl], acc[:], pWT[:, m, n_sl], op=SUB
                        )
            st[b]["pWT"], st[b]["pWTl"] = pWT, pWTl

        def phase_mm2sm(b):
            """scores into PSUM; softmax straight out of PSUM into fp16 wT."""
            qT = st[b]["qT"]
            pWT = st[b]["pWT"]
            wT = wt_pool.tile([P, C, t], f16, name=f"wT_{b}", tag="wT")
            negmax = stats.tile([P, C, TH], f32, name=f"negmax_{b}", tag="negmax")
            nm = stats.tile([P, C], f32, name=f"nm_{b}", tag="nm")
            sume = stats.tile([P, C, TH], f32, name=f"sume_{b}", tag="sume")
            recip = stats.tile([P, C], f32, name=f"recip_{b}", tag="recip")
            for m in range(C):
                msl = slice(m * P, (m + 1) * P)
                accs = []
                for n in range(TH):
                    n_sl = slice(n * H, (n + 1) * H)
                    acc = psum_mm.tile([P, H], f32, name=f"a2_{b}_{m}_{n}", tag="acc")

                    def qt_sl(k, msl=msl, qT=qT):
                        return qT[:, k, msl]

                    terms = [(qt_sl, pWT)]
                    if split2:
                        qTl = st[b]["qTl"]
                        terms += [
                            (qt_sl, st[b]["pWTl"]),
                            (lambda k, msl=msl, qTl=qTl: qTl[:, k, msl], pWT),
                        ]
                    mm_terms(acc, terms, n_sl)
                    nc.vector.reduce_max(
                        negmax[:, m, n : n + 1], acc[:], axis=AX, negate=True
                    )
                    accs.append(acc)
                if TH > 1:
                    nc.vector.tensor_tensor(
                        nm[:, m : m + 1], negmax[:, m, 0:1], negmax[:, m, 1:2], op=MIN
                    )
                    nm_sl = nm[:, m : m + 1]
                else:
                    nm_sl = negmax[:, m, 0:1]
                for n, acc in enumerate(accs):
                    nc.scalar.activation(
                        wT[:, m, n * H : (n + 1) * H],
                        acc[:],
                        EXP,
                        bias=nm_sl,
                        accum_out=sume[:, m, n : n + 1],
                    )
                if TH > 1:
                    nc.vector.tensor_tensor(
                        recip[:, m : m + 1], sume[:, m, 0:1], sume[:, m, 1:2], op=ADD
                    )
                    nc.vector.reciprocal(recip[:, m : m + 1], recip[:, m : m + 1])
                else:
                    nc.vector.reciprocal(recip[:, m : m + 1], sume[:, m, 0:1])
                nc.vector.tensor_scalar_mul(wT[:, m, :], wT[:, m, :], recip[:, m : m + 1])
            st[b]["wT"] = wT

        def phase_mm3(b):
            """out[tp, d] = sum_tq wT[tq,tp] * qh[tq,d]."""
            wT = st[b]["wT"]
            qh = st[b]["qh"]
            for m in range(C):
                msl = slice(m * P, (m + 1) * P)
                for n in range(NH):
                    n_sl = slice(n * H, (n + 1) * H)
                    acc = psum_mm.tile([P, H], f32, name=f"a3_{b}_{m}_{n}", tag="acc")
                    mm_terms(
                        acc, [(lambda k, msl=msl: wT[:, k, msl], qh)], n_sl
                    )
                    ot = ostage.tile([P, H], f32, name=f"ot_{b}_{m}_{n}", tag="ot")
                    nc.scalar.copy(ot[:], acc[:])
                    nc.sync.dma_start(
                        out_ext[b, m * P : (m + 1) * P, n * H : (n + 1) * H], ot[:]
                    )

        # Emission order = per-engine program order. Batch b+1's mm1 is
        # emitted before batch b's mm3 so the PE stays busy while b's softmax
        # tail completes.
        phase_loads(0)
        phase_mm1(0)
        for b in range(b_loc):
            phase_mm2sm(b)
            if b + 1 < b_loc:
                phase_loads(b + 1)
                phase_mm1(b + 1)
            phase_mm3(b)

    nc.finalize()  # run the Bacc legalization/regalloc passes for walrus
    return nc


_CACHE = {}


def _get_nc(mode=MODE):
    key = mode
    if key not in _CACHE:
        _CACHE[key] = build_nc(B_FULL // N_CORES, T_FULL, D_FULL, mode=mode)
    return _CACHE[key]


def _prep_inputs(q, p, W, mode=MODE):
    """Host-side layout prep: fp16 casts (+ residuals for split mode) and
    per-batch transposes of q and p."""
    q = np.ascontiguousarray(q, dtype=np.float32)
    p = np.ascontiguousarray(p, dtype=np.float32)
    W = np.ascontiguousarray(W, dtype=np.float32)
    m1, m2 = mode
    d = W.shape[0]
    KC = d // P

    def block_w(x16):
        # [d, d] -> [m, p, ce, c] with x[ce*128+p, m*128+c]
        return np.ascontiguousarray(
            x16.reshape(KC, P, KC, P).transpose(2, 1, 0, 3)
        )

    t = q.shape[1]
    C = t // P
    H = 512
    TH = t // H

    def pack(x16):
        # [b, t, cols] -> [b, p, c, cols]: 16KB contiguous per partition
        b, _, cols = x16.shape
        return np.ascontiguousarray(
            x16.reshape(b, C, P, cols).transpose(0, 2, 1, 3)
        )

    def pack_halved(x16):
        # [b, t, cols] -> [b, h, p, c, H]: 8KB contiguous per partition
        b, _, cols = x16.shape
        return np.ascontiguousarray(
            x16.reshape(b, C, P, TH, H).transpose(0, 3, 2, 1, 4)
        )

    qh = q.astype(np.float16)
    qt = np.transpose(qh, (0, 2, 1))
    pt = np.transpose(p, (0, 2, 1)).astype(np.float16)
    wh = W.astype(np.float16)
    arrs = {
        "qh": pack(qh),
        "qt": pack(qt),
        "pt": pack_halved(pt),
        "w": block_w(wh),
    }
    if m2 == "split":
        qtf = np.transpose(q, (0, 2, 1))
        arrs["qtl"] = pack((qtf - qt.astype(np.float32)).astype(np.float16))
    if m1 == "split":
        ptf = np.transpose(p, (0, 2, 1))
        arrs["ptl"] = pack((ptf - pt.astype(np.float32)).astype(np.float16))
        arrs["wl"] = block_w((W - wh.astype(np.float32)).astype(np.float16))
    return arrs


def run(q, p, W, mode=MODE, nc=None, **spmd_kwargs):
    """Run on 8 NeuronCores; returns (out, BassKernelResults)."""
    from concourse.bass_utils import run_bass_kernel_spmd

    arrs = _prep_inputs(q, p, W, mode=mode)
    if nc is None:
        nc = _get_nc(mode)
    bl = B_FULL // N_CORES
    batch_sharded = {"qh", "qt", "pt", "qtl", "ptl"}
    in_maps = []
    for i in range(N_CORES):
        m = {}
        for name, a in arrs.items():
            m[name] = a[i * bl : (i + 1) * bl] if name in batch_sharded else a
        in_maps.append(m)
    res = run_bass_kernel_spmd(nc, in_maps, list(range(N_CORES)), **spmd_kwargs)
    out = np.concatenate([res.results[i]["out"] for i in range(N_CORES)], axis=0)
    return out, res


def kernel(q, p, W):
    out, _ = run(q, p, W)
    return out
